# revision 14
# baseline (speedup 1.0000x reference)
"""GAT layer (nn_GATLayer) on 8 Trainium2 NeuronCores.

Math (per batch b):
    h   = x @ W                      [N, D]
    s1  = h @ a1   (free-dim i)      [N]
    s2  = h @ a2   (partition j)     [N]
    e   = lrelu(s1_i + s2_j)  masked by adj[i, j], softmax over j
    out = attn @ h

Device formulation (per core = one batch element), in [p=j, f=i] layout:
    PT[j, i] = exp(0.2 * max(y, 5y)),  y = s1[i] + s2[j] + maskbias[j, i]
      (lrelu(x) = 0.2*max(5x, x); maskbias = 0 or -1e9 pre-lrelu, exp -> 0)
    numT[d, i] = sum_j h_cat[j, d] * PT[j, i],  h_cat = [h | ones]  (bf16)
    out[i, d]  = numT[d, i] / numT[64, i]

Sharding: data-parallel over batch B=8 across the 8 cores. Host prep:
x[b] transposed to xT [64, 2048]; maskbias = where(adj.T>0, 0, -1e9) bf16
(shared across cores).

Constraint honored throughout: fp32 (self-loading) PE instructions can carry
at most ONE sync wait in walrus codegen (S3_LW struct), so every tile read by
a PE op is staged through the ACT engine (single semaphore), and a dummy PE
matmul absorbs the xT DMA-queue wait up front.
"""

import os
import sys

sys.path.insert(0, "/opt/trn_rl_repo")

import numpy as np
import ml_dtypes

B, N, DIN, DOUT = 8, 2048, 64, 64
NCORES = 8
PJ = 128              # j-tile partition size
NJT = N // PJ         # 16 j-tiles
FCH = 512             # psum bank chunk (fp32)
NCH = N // FCH        # 4 chunks of the free dim
NEG_BIG = -1.0e9
HCAT_STRIDE = 66      # 64 h cols + 1 ones col + 1 pad
EPI_GRP = 4           # epilogue transposes packed per psum bank tile

_GAT_OP = None
_COMPILED = None
LAST_RESULT = None    # BassKernelResults from the last run (for test.py)


def _register_gat_op():
    """Fused score op:  out = max(y, y*imm2), y = (in0 + s0) + in1.

    in0 = s1 broadcast [128, N] (f32), s0 = s2 per-partition [128, 1] (f32),
    in1 = maskbias tile [128, N] (bf16), imm2 = 5.0.
    """
    global _GAT_OP
    if _GAT_OP is not None:
        return _GAT_OP
    from concourse.dve_ops import (
        OPS,
        CUSTOM_DVE_SPECS,
        DveOp,
        _SUB_OPCODE_FOR_NAME,
    )
    from concourse.dve_spec import Spec, Src0, Src1, C0, C2, maxx, lower, _has_src1
    from concourse.dve_uop import DveOpSpec

    name = "GAT_SCORE_ANT"
    if name in _SUB_OPCODE_FOR_NAME:
        _GAT_OP = next(op for op in OPS if op.name == name)
        return _GAT_OP

    _y = (Src0 + C0) + Src1

    def _ref(in0, in1, s0, s1, imm2):
        y = (in0.astype(np.float32) + s0) + in1.astype(np.float32)
        return np.maximum(y, y * imm2).astype(np.float32)

    spec = Spec(body=maxx(_y, _y * C2), reference=_ref)
    row = max(_SUB_OPCODE_FOR_NAME.values()) + 1
    assert row < 0x20
    _SUB_OPCODE_FOR_NAME[name] = row
    shas = {}
    for ver in ("v3", "v4"):
        tmp = DveOpSpec(
            name=name, opcode=row, uops=lower(spec, ver=ver), rd1_en=_has_src1(spec)
        )
        shas[ver] = tmp.sha(ver)
    op = DveOp(name, spec, subdim=False, uops_sha=shas)
    OPS.append(op)
    CUSTOM_DVE_SPECS[name] = spec
    _GAT_OP = op
    return op


def _build_nc():
    """Build the Bass module (shared SPMD program for all 8 cores)."""
    from contextlib import ExitStack

    import concourse.bass as bass
    import concourse.tile as tile
    from concourse import bacc, masks, mybir

    gat_op = _register_gat_op()

    f32 = mybir.dt.float32
    bf16 = mybir.dt.bfloat16
    AF = mybir.ActivationFunctionType

    nc = bacc.Bacc("TRN2", target_bir_lowering=False, debug=False, num_devices=NCORES)

    xT = nc.dram_tensor("xt", [DIN, N], f32, kind="ExternalInput").ap()
    mb = nc.dram_tensor("maskbias", [N, N], bf16, kind="ExternalInput").ap()
    w = nc.dram_tensor("w", [DIN, DOUT], f32, kind="ExternalInput").ap()
    a1 = nc.dram_tensor("a1", [DOUT, 1], f32, kind="ExternalInput").ap()
    a2 = nc.dram_tensor("a2", [DOUT, 1], f32, kind="ExternalInput").ap()
    out = nc.dram_tensor("out", [N, DOUT], f32, kind="ExternalOutput").ap()

    with ExitStack() as ctx:
        tc = ctx.enter_context(tile.TileContext(nc))

        const = ctx.enter_context(tc.tile_pool(name="const", bufs=1))
        big = ctx.enter_context(tc.tile_pool(name="big", bufs=1))

        # ---- inputs to SBUF ----
        w_dma = const.tile([DIN, DOUT], f32, tag="w0")
        nc.sync.dma_start(w_dma[:], w)
        a1_dma = const.tile([DOUT, 1], f32, tag="a10")
        nc.sync.dma_start(a1_dma[:], a1)
        a2_dma = const.tile([DOUT, 1], f32, tag="a20")
        nc.sync.dma_start(a2_dma[:], a2)
        xT_sb = const.tile([DIN, N], f32, tag="xt")
        nc.sync.dma_start(xT_sb[:], xT)

        # ACT-staged copies: every tile a PE instruction reads is written by
        # the ACT engine, so each fp32 matmul needs at most one sync wait.
        w_sb = const.tile([DIN, DOUT], f32, tag="w")
        nc.scalar.copy(w_sb[:], w_dma[:])
        a2_sb = const.tile([DOUT, 1], f32, tag="a2")
        nc.scalar.copy(a2_sb[:], a2_dma[:])
        a1rep = const.tile([DOUT, PJ], f32, tag="a1rep")
        nc.scalar.copy(a1rep[:], a1_dma[:].broadcast_to([DOUT, PJ]))

        ident0 = const.tile([PJ, PJ], f32, tag="ident0")
        masks.make_identity(nc, ident0[:])
        ident = const.tile([PJ, PJ], f32, tag="ident")
        nc.scalar.copy(ident[:], ident0[:])

        ones_sb = const.tile([PJ, 1], bf16, tag="ones")
        nc.vector.memset(ones_sb[:], 1.0)

        hT_sb = big.tile([DIN, N], f32, tag="ht")      # h^T
        s1b_sb = big.tile([PJ, N], f32, tag="s1b")     # s1 broadcast to 128 rows
        s2_all = big.tile([PJ, NJT], f32, tag="s2")    # s2, col jt = j-tile chunk
        hcat = big.tile([PJ, NJT * HCAT_STRIDE], bf16, tag="hcat")  # [h | 1]

        # ones columns of h_cat via one strided ACT copy
        hcat3 = hcat[:].rearrange("p (t s) -> p t s", s=HCAT_STRIDE)
        nc.scalar.copy(
            hcat3[:, :, DOUT : DOUT + 1],
            ones_sb[:].broadcast_to([PJ, NJT])[:, :, None],
        )


        # ---- prologue: h^T; s1b; s2; h_cat ----
        with tc.tile_pool(name="pro_psum", bufs=1, space="PSUM") as ppool:
            hT_ps = ppool.tile([DIN, N], f32, tag="ht_ps")
            for c in range(NCH):
                sl = slice(c * FCH, (c + 1) * FCH)
                nc.tensor.matmul(
                    hT_ps[:, sl], w_sb[:], xT_sb[:, sl], start=True, stop=True
                )
                nc.scalar.copy(hT_sb[:, sl], hT_ps[:, sl])

        with tc.tile_pool(name="pro2_psum", bufs=1, space="PSUM") as ppool2, \
             tc.tile_pool(name="s2_psum", bufs=1, space="PSUM") as spool, \
             tc.tile_pool(name="htr_psum", bufs=2, space="PSUM") as ppool3:
            # s1b[p, i] = s1[i] for every p: lhsT = a1rep [64, 128], rhs = hT
            s1b_ps = ppool2.tile([PJ, N], f32, tag="s1b_ps")
            for c in range(NCH):
                sl = slice(c * FCH, (c + 1) * FCH)
                nc.tensor.matmul(
                    s1b_ps[:, sl], a1rep[:], hT_sb[:, sl], start=True, stop=True
                )
                nc.scalar.copy(s1b_sb[:, sl], s1b_ps[:, sl])

            # s2 chunks: lhsT = hT[:, jt*128:...], rhs = a2  -> [128, 1]
            s2_ps = spool.tile([PJ, NJT], f32, tag="s2_ps")
            for jt in range(NJT):
                jsl = slice(jt * PJ, (jt + 1) * PJ)
                nc.tensor.matmul(
                    s2_ps[:, jt : jt + 1], hT_sb[:, jsl], a2_sb[:],
                    start=True, stop=True,
                )
            nc.scalar.copy(s2_all[:], s2_ps[:])

            # h tiles: PE-transpose hT chunks, pack 8 per psum bank, cast bf16
            for half in range(2):
                htr_ps = ppool3.tile([PJ, 8 * DOUT], f32, tag="htr")
                for k in range(8):
                    jt = half * 8 + k
                    jsl = slice(jt * PJ, (jt + 1) * PJ)
                    nc.tensor.transpose(
                        htr_ps[:, k * DOUT : (k + 1) * DOUT],
                        hT_sb[:, jsl],
                        ident[:DIN, :DIN],
                    )
                dst = hcat3[:, half * 8 : (half + 1) * 8, :DOUT]
                src = htr_ps[:].rearrange("p (t s) -> p t s", s=DOUT)
                nc.scalar.copy(dst, src)

        # ---- main loop over j-tiles ----
        mpool = ctx.enter_context(tc.tile_pool(name="mask", bufs=3))
        tpool = ctx.enter_context(tc.tile_pool(name="scores", bufs=2))
        ppool_e = ctx.enter_context(tc.tile_pool(name="probs", bufs=2))
        num_pool = ctx.enter_context(
            tc.tile_pool(name="num_psum", bufs=1, space="PSUM")
        )

        numT_ps = num_pool.tile([DOUT + 1, N], f32, tag="numt")

        for jt in range(NJT):
            mb_sb = mpool.tile([PJ, N], bf16, tag="mb")
            nc.sync.dma_start(mb_sb[:], mb[jt * PJ : (jt + 1) * PJ, :])

            t_sb = tpool.tile([PJ, N], f32, tag="t")
            nc.vector._custom_dve(
                gat_op,
                out=t_sb[:],
                in0=s1b_sb[:],
                in1=mb_sb[:],
                s0=s2_all[:, jt : jt + 1],
                s1=0.0,
                imm2=5.0,
            )

            p_sb = ppool_e.tile([PJ, N], bf16, tag="p")
            nc.scalar.activation(p_sb[:], t_sb[:], AF.Exp, scale=0.2)

            lhsT = hcat[:, jt * HCAT_STRIDE : jt * HCAT_STRIDE + DOUT + 1]
            for c in range(NCH):
                sl = slice(c * FCH, (c + 1) * FCH)
                nc.tensor.matmul(
                    numT_ps[:, sl], lhsT, p_sb[:, sl],
                    start=(jt == 0), stop=(jt == NJT - 1),
                )

        # ---- epilogue: transpose numT, divide by row-sums, store ----
        epool = ctx.enter_context(tc.tile_pool(name="epi", bufs=2))
        etr_pool = ctx.enter_context(
            tc.tile_pool(name="epi_psum", bufs=2, space="PSUM")
        )
        out_pool = ctx.enter_context(tc.tile_pool(name="out", bufs=1))

        numT_sb = big.tile([DOUT + 1, N], f32, tag="numt_sb")
        nc.scalar.copy(numT_sb[:], numT_ps[:])

        out_sb = out_pool.tile([PJ, NJT * DOUT], f32, tag="out")
        GW = EPI_GRP * (DOUT + 1)  # grouped transpose width per psum tile
        for g in range(NJT // EPI_GRP):
            tr_ps = etr_pool.tile([PJ, GW], f32, tag="tr")
            for k in range(EPI_GRP):
                it = g * EPI_GRP + k
                isl = slice(it * PJ, (it + 1) * PJ)
                nc.tensor.transpose(
                    tr_ps[:, k * (DOUT + 1) : (k + 1) * (DOUT + 1)],
                    numT_sb[:, isl],
                    ident[: DOUT + 1, : DOUT + 1],
                )
            # single ACT drain per group keeps the PSUM slot reader on ACT
            tr_sb = epool.tile([PJ, GW], f32, tag="tr_sb")
            nc.scalar.copy(tr_sb[:], tr_ps[:])

            tr3 = tr_sb[:].rearrange("p (k s) -> p k s", s=DOUT + 1)
            recip = epool.tile([PJ, EPI_GRP], f32, tag="recip")
            nc.vector.reciprocal(recip[:], tr3[:, :, DOUT])
            for k in range(EPI_GRP):
                it = g * EPI_GRP + k
                nc.vector.tensor_scalar_mul(
                    out_sb[:, it * DOUT : (it + 1) * DOUT],
                    tr3[:, k, :DOUT],
                    recip[:, k : k + 1],
                )

        out_3d = out.rearrange("(t p) d -> p t d", p=PJ)
        nc.sync.dma_start(out_3d, out_sb[:].rearrange("p (t d) -> p t d", d=DOUT))

    nc.compile()
    return nc


def _prep_inputs(x, adj, W, a):
    xT = np.ascontiguousarray(np.transpose(x, (0, 2, 1)), dtype=np.float32)
    mask_bias = np.where(adj.T > 0, np.float32(0.0), np.float32(NEG_BIG)).astype(
        ml_dtypes.bfloat16
    )
    a = np.asarray(a, dtype=np.float32)
    a1 = np.ascontiguousarray(a[:DOUT].reshape(DOUT, 1))
    a2 = np.ascontiguousarray(a[DOUT:].reshape(DOUT, 1))
    W = np.ascontiguousarray(np.asarray(W, dtype=np.float32))
    in_maps = []
    for b in range(NCORES):
        in_maps.append(
            {
                "xt": xT[b],
                "maskbias": mask_bias,
                "w": W,
                "a1": a1,
                "a2": a2,
            }
        )
    return in_maps


def kernel(x, adj, W, a):
    global _COMPILED, LAST_RESULT
    from concourse import bass_utils

    x = np.asarray(x)
    adj = np.asarray(adj)
    assert x.shape == (B, N, DIN) and adj.shape == (N, N)

    if _COMPILED is None:
        _COMPILED = _build_nc()
    nc = _COMPILED

    in_maps = _prep_inputs(x, adj, W, a)
    res = bass_utils.run_bass_kernel_spmd(
        nc,
        in_maps,
        core_ids=list(range(NCORES)),
        trace=bool(int(os.environ.get("GAT_TRACE", "0"))),
    )
    LAST_RESULT = res
    out = np.stack([res.results[c]["out"] for c in range(NCORES)], axis=0)
    return out.astype(np.float32)
